# revision 1
# baseline (speedup 1.0000x reference)
"""AdaptivePoolingAttention on 8 TRN2 NeuronCores.

Data-parallel over segments: each core owns 4 of the 32 segments (attention is
block-diagonal per segment), weights replicated. No collectives.

Host wrapper pre-packs layouts (pure data movement + bf16 rounding):
  xT       [D, TOK]  bf16 per core (x transposed)
  wq_pack  [H, 128, D]      bf16: wq_pack[h, p, dd*128+c] = wq[dd*128+p, h*128+c]
  wkv_pack [KV/128, 128, D] bf16: same per 128-col tile
  wo_pack  [D/NJ, 128, H*NJ] bf16: wo_pack[j, p, hh*NJ+c] = wo[hh*128+p, j*NJ+c]

Device pipeline (bf16 matmuls, fp32 PSUM accumulation):
  A: DMA xT resident in SBUF; pooled queries = strided mean over xT free dim,
     transposed back to natural layout via TensorE.
  B: rmsnorm(queries), transpose -> qnT; spill queries to DRAM.
  C: qT = wq-proj(qnT) scaled by HD**-0.5.
  D: kvT = wkv^T @ xT (8192 N=512 matmuls), spilled to DRAM scratch.
  E: per (seg, head): scores, softmax (no max-sub; scores bounded), TensorE
     transposes of attn and vT, oT accumulation.
  F: out = oT' @ wo + queries, final rmsnorm.
"""

import sys

sys.path.insert(0, "/opt/trn_rl_repo")

import numpy as np
from contextlib import ExitStack

SEG, L, R, POOL, H, HD, D, EPS = 32, 512, 64, 8, 32, 128, 4096, 1e-5
NCORES = 8
SEGC = SEG // NCORES          # 4 segments per core
TOK = SEGC * L                # 2048 tokens per core
Q = SEGC * R                  # 256 queries per core
KV = 2 * H * HD               # 8192
NJ = 512                      # wo col-tile width
ND = D // 128                 # 32 contraction tiles

_CACHE = {}


def _build():
    import concourse.bass as bass
    import concourse.mybir as mybir
    import concourse.tile as tile
    from concourse import bacc

    f32 = mybir.dt.float32
    bf16 = mybir.dt.bfloat16
    ts = bass.ts
    ds = bass.ds
    AF = mybir.ActivationFunctionType
    ALU = mybir.AluOpType

    nc = bacc.Bacc("TRN2", target_bir_lowering=False, debug=False)

    xT_e = nc.declare_dram_parameter("xT", [D, TOK], bf16, isOutput=False)
    wq_e = nc.declare_dram_parameter("wq_pack", [H, 128, D], bf16, isOutput=False)
    wkv_e = nc.declare_dram_parameter(
        "wkv_pack", [KV // 128, 128, D], bf16, isOutput=False
    )
    wo_e = nc.declare_dram_parameter(
        "wo_pack", [D // NJ, 128, H * NJ], bf16, isOutput=False
    )
    anw_e = nc.declare_dram_parameter("attn_norm_w", [128, D], bf16, isOutput=False)
    onw_e = nc.declare_dram_parameter("out_norm_w", [128, D], bf16, isOutput=False)
    id_e = nc.declare_dram_parameter("ident", [128, 128], bf16, isOutput=False)
    out_e = nc.declare_dram_parameter("out", [Q, D], f32, isOutput=True)

    kT_d = nc.dram_tensor("kT_scratch", [H * 128, TOK], bf16)   # 16 MiB
    vT_d = nc.dram_tensor("vT_scratch", [H * 128, TOK], bf16)   # 16 MiB
    qspill_d = nc.dram_tensor("q_spill", [2, 128, D], bf16)      # 2 MiB

    with tile.TileContext(nc, pool_alloc_mode="queue") as tc, ExitStack() as st:
        # ---- constants ------------------------------------------------
        cst = st.enter_context(tc.tile_pool(name="const", bufs=1))
        ident = cst.tile([128, 128], bf16)
        epst = cst.tile([128, 1], f32)
        nc.vector.memset(epst[:], EPS)
        nc.sync.dma_start(ident[:], id_e[:])

        # qT persists C..E (16 KiB/part)
        qTp = st.enter_context(tc.tile_pool(name="qTp", bufs=1))
        qT = qTp.tile([128, H, Q], bf16)

        # ---- xT lives A..D (128 KiB/part) -----------------------------
        with tc.tile_pool(name="xTp", bufs=1) as xT_p:
            xT = xT_p.tile([128, ND, TOK], bf16)
            for s in range(SEGC):
                for dblk in range(ND):
                    nc.sync.dma_start(
                        xT[:, dblk, ts(s, 512)],
                        xT_e[ds(dblk * 128, 128), ts(s, 512)],
                    )

            with tc.tile_pool(name="qnT", bufs=1) as qnT_p:
                qnT = qnT_p.tile([128, ND, Q], bf16)
                with tc.tile_pool(name="qn", bufs=1) as qn_p:
                    q_nat = [
                        qn_p.tile([128, D], bf16, tag=f"qnat{i}", name=f"qnat{i}")
                        for i in range(2)
                    ]
                    # ---- stage A: pooled queries from xT --------------
                    with (
                        tc.tile_pool(name="pstg", bufs=3) as pstg_p,
                        tc.tile_pool(name="pq", bufs=3, space="PSUM") as pq_ps,
                    ):
                        for dblk in range(ND):
                            qtrf = pstg_p.tile([128, Q], f32, tag="qtrf")
                            nc.vector.tensor_reduce(
                                qtrf[:],
                                xT[:, dblk, :].rearrange(
                                    "p (q e) -> p q e", e=POOL
                                ),
                                axis=mybir.AxisListType.X, op=ALU.add,
                            )
                            qtrb = pstg_p.tile([128, Q], bf16, tag="qtrb")
                            nc.vector.tensor_scalar_mul(
                                qtrb[:], qtrf[:], 1.0 / POOL
                            )
                            for qt in range(2):
                                pt2 = pq_ps.tile([128, 128], bf16, tag="pt2")
                                nc.tensor.transpose(
                                    pt2[:], qtrb[:, ts(qt, 128)], ident[:]
                                )
                                nc.any.tensor_copy(
                                    q_nat[qt][:, ts(dblk, 128)], pt2[:]
                                )

                    # ---- stage B: rmsnorm(queries) -> qnT -------------
                    with (
                        tc.tile_pool(name="bsc", bufs=1) as bsc_p,
                        tc.tile_pool(name="bps", bufs=4, space="PSUM") as bps,
                    ):
                        bw_attn = bsc_p.tile([128, D], bf16, tag="bwa")
                        nc.sync.dma_start(bw_attn[:], anw_e[:])
                        for qt in range(2):
                            nc.sync.dma_start(qspill_d[qt], q_nat[qt][:])
                            qnn = bsc_p.tile([128, D], bf16, tag="qnn")
                            ssq = bsc_p.tile([128, 1], f32, tag="ssq")
                            nc.scalar.activation(
                                qnn[:], q_nat[qt][:], AF.Square,
                                accum_out=ssq[:],
                            )
                            srt = bsc_p.tile([128, 1], f32, tag="srt")
                            nc.scalar.activation(
                                srt[:], ssq[:], AF.Sqrt,
                                bias=epst[:], scale=1.0 / D,
                            )
                            rs = bsc_p.tile([128, 1], f32, tag="rs")
                            nc.vector.reciprocal(rs[:], srt[:])
                            nc.vector.tensor_scalar_mul(
                                qnn[:], q_nat[qt][:], rs[:]
                            )
                            nc.vector.tensor_tensor(
                                qnn[:], qnn[:], bw_attn[:], op=ALU.mult
                            )
                            for dblk in range(ND):
                                pt = bps.tile([128, 128], bf16)
                                nc.tensor.transpose(
                                    pt[:], qnn[:, ts(dblk, 128)], ident[:]
                                )
                                nc.any.tensor_copy(
                                    qnT[:, dblk, ts(qt, 128)], pt[:]
                                )

                # ---- stage C: qT = wq-proj(qnT) -----------------------
                with (
                    tc.tile_pool(name="wqb", bufs=3) as wqb_p,
                    tc.tile_pool(name="cps", bufs=2, space="PSUM") as cps,
                ):
                    for h in range(H):
                        wqb = wqb_p.tile([128, ND, 128], bf16)
                        nc.scalar.dma_start(
                            wqb[:],
                            wq_e[h].rearrange("p (dd c) -> p dd c", c=128),
                        )
                        psq = cps.tile([128, Q], f32)
                        for dblk in range(ND):
                            nc.tensor.matmul(
                                psq[:], wqb[:, dblk, :], qnT[:, dblk, :],
                                start=(dblk == 0), stop=(dblk == ND - 1),
                            )
                        nc.scalar.mul(qT[:, h, :], psq[:], float(HD) ** -0.5)

            # ---- stage D: kvT = wkv^T @ xT, spill ---------------------
            with (
                tc.tile_pool(name="wkb", bufs=3) as wkb_p,
                tc.tile_pool(name="kst", bufs=3) as kst_p,
                tc.tile_pool(name="dps", bufs=2, space="PSUM") as dps,
            ):
                for c in range(KV // 128):
                    wkb = wkb_p.tile([128, ND, 128], bf16)
                    nc.sync.dma_start(
                        wkb[:],
                        wkv_e[c].rearrange("p (dd cc) -> p dd cc", cc=128),
                    )
                    pss = [
                        dps.tile([128, 512], f32, tag=f"dps{sq}", name=f"dps{sq}")
                        for sq in range(SEGC)
                    ]
                    for dblk in range(ND):
                        for sq in range(SEGC):
                            nc.tensor.matmul(
                                pss[sq][:], wkb[:, dblk, :],
                                xT[:, dblk, ts(sq, 512)],
                                start=(dblk == 0), stop=(dblk == ND - 1),
                            )
                    for sq in range(SEGC):
                        kb = kst_p.tile([128, 512], bf16)
                        nc.any.tensor_copy(kb[:], pss[sq][:])
                        if c < H:
                            nc.sync.dma_start(
                                kT_d[ts(c, 128), ts(sq, 512)], kb[:]
                            )
                        else:
                            nc.sync.dma_start(
                                vT_d[ts(c - H, 128), ts(sq, 512)], kb[:]
                            )

        # ---- stage E: attention (xT freed; oT spans E..F) -------------
        with tc.tile_pool(name="oTp", bufs=1) as oT_p:
            oT = oT_p.tile([128, H, Q], bf16)
            with (
                tc.tile_pool(name="kv_in", bufs=2) as kvin_p,
                tc.tile_pool(name="att", bufs=4) as att_p,
                tc.tile_pool(name="sps", bufs=2, space="PSUM") as sps,
                tc.tile_pool(name="tps", bufs=2, space="PSUM") as tps,
                tc.tile_pool(name="ops", bufs=2, space="PSUM") as ops,
                tc.tile_pool(name="wob", bufs=2) as wob_p,
                tc.tile_pool(name="qrl", bufs=1) as qrl_p,
                tc.tile_pool(name="fout", bufs=1) as fout_p,
                tc.tile_pool(name="fsc", bufs=1) as fsc_p,
                tc.tile_pool(name="fps", bufs=2, space="PSUM") as fps,
            ):
                for s in range(SEGC):
                    kts_half = {}
                    for half in range(2):
                        kth = kvin_p.tile(
                            [128, H // 2, 512], bf16, tag="kTs", name="kth"
                        )
                        nc.sync.dma_start(
                            kth[:],
                            kT_d[ds(half * (H // 2) * 128, (H // 2) * 128),
                                 ts(s, 512)].rearrange("(h p) t -> p h t", p=128),
                        )
                        kts_half[half] = kth
                    for hp in range(H // 2):
                        kTs = kts_half[hp // 8]
                        hb = (hp % 8) * 2
                        h0, h1 = 2 * hp, 2 * hp + 1
                        scp = sps.tile([128, 512], f32)
                        nc.tensor.matmul(
                            scp[:64, :], qT[:, h0, ds(s * R, R)],
                            kTs[:, hb, :],
                            start=True, stop=True,
                        )
                        nc.tensor.matmul(
                            scp[64:, :], qT[:, h1, ds(s * R, R)],
                            kTs[:, hb + 1, :],
                            start=True, stop=True,
                            tile_position=(0, 64),
                        )
                        esb = att_p.tile([128, 512], bf16, tag="esb")
                        nc.scalar.activation(esb[:], scp[:], AF.Exp)
                        rsum = att_p.tile([128, 1], f32, tag="rsum")
                        nc.vector.tensor_reduce(
                            rsum[:], esb[:], axis=mybir.AxisListType.X, op=ALU.add
                        )
                        rrec = att_p.tile([128, 1], f32, tag="rrec")
                        nc.vector.reciprocal(rrec[:], rsum[:])
                        attn = att_p.tile([128, 512], bf16, tag="attn")
                        nc.vector.tensor_scalar_mul(attn[:], esb[:], rrec[:])
                        aT = att_p.tile([128, 4, 128], bf16, tag="aT")
                        vN = att_p.tile([128, 2, 4, 128], bf16, tag="vN")
                        for lb in range(4):
                            pa = tps.tile([128, 128], bf16, tag="pa")
                            nc.tensor.transpose(
                                pa[:], attn[:, ts(lb, 128)], ident[:]
                            )
                            nc.any.tensor_copy(aT[:, lb, :], pa[:])
                        for hi, h in enumerate((h0, h1)):
                            for lb in range(4):
                                nc.scalar.dma_start(
                                    vN[:, hi, lb, :],
                                    vT_d[ts(h, 128), ds(s * 512 + lb * 128, 128)],
                                    transpose=True,
                                )
                            po = ops.tile([128, 64], f32)
                            for lb in range(4):
                                nc.tensor.matmul(
                                    po[:], vN[:, hi, lb, :],
                                    aT[:, lb, ds(hi * 64, 64)],
                                    start=(lb == 0), stop=(lb == 3),
                                )
                            nc.any.tensor_copy(oT[:, h, ds(s * R, R)], po[:])

                # ---- stage F (shares E scope for overlap) -------------
                q_rl = [
                    qrl_p.tile([128, D], bf16, tag=f"qrl{i}", name=f"qrl{i}")
                    for i in range(2)
                ]
                for qt in range(2):
                    nc.sync.dma_start(q_rl[qt][:], qspill_d[qt])
                bw_out = fsc_p.tile([128, D], bf16, tag="bwo")
                nc.sync.dma_start(bw_out[:], onw_e[:])
                out_sb = [
                    fout_p.tile([128, D], bf16, tag=f"osb{i}", name=f"osb{i}")
                    for i in range(2)
                ]
                for j in range(D // NJ):
                    wob = wob_p.tile([128, H, NJ], bf16, tag="wob", name="wob")
                    nc.sync.dma_start(
                        wob[:],
                        wo_e[j].rearrange("p (hh c) -> p hh c", c=NJ),
                    )
                    for qt in range(2):
                        ps = fps.tile([128, NJ], f32)
                        for hh in range(H):
                            nc.tensor.matmul(
                                ps[:], oT[:, hh, ts(qt, 128)], wob[:, hh, :],
                                start=(hh == 0), stop=(hh == H - 1),
                            )
                        nc.vector.tensor_tensor(
                            out_sb[qt][:, ts(j, NJ)], ps[:],
                            q_rl[qt][:, ts(j, NJ)], op=ALU.add,
                        )
                for qt in range(2):
                    scrh = fsc_p.tile([128, D // 2], bf16, tag="fscr")
                    ssq = fsc_p.tile([128, 1], f32, tag="fssq")
                    ssqa = fsc_p.tile([128, 1], f32, tag="fssqa")
                    for half in range(2):
                        nc.scalar.activation(
                            scrh[:], out_sb[qt][:, ts(half, D // 2)],
                            AF.Square, accum_out=(ssq if half else ssqa)[:],
                        )
                    nc.vector.tensor_tensor(
                        ssq[:], ssq[:], ssqa[:], op=ALU.add
                    )
                    srt = fsc_p.tile([128, 1], f32, tag="fsrt")
                    nc.scalar.activation(
                        srt[:], ssq[:], AF.Sqrt, bias=epst[:], scale=1.0 / D
                    )
                    rs = fsc_p.tile([128, 1], f32, tag="frs")
                    nc.vector.reciprocal(rs[:], srt[:])
                    for half in range(2):
                        fin = fsc_p.tile([128, D // 2], f32, tag="ffin")
                        nc.vector.tensor_scalar_mul(
                            fin[:], out_sb[qt][:, ts(half, D // 2)], rs[:]
                        )
                        nc.vector.tensor_tensor(
                            fin[:], fin[:], bw_out[:, ts(half, D // 2)],
                            op=ALU.mult,
                        )
                        nc.sync.dma_start(
                            out_e[ts(qt, 128), ts(half, D // 2)], fin[:]
                        )

    nc.finalize()
    return nc


def _in_maps(inputs):
    import ml_dtypes

    bf = ml_dtypes.bfloat16
    x = np.asarray(inputs["x"], dtype=np.float32)
    wq = np.asarray(inputs["wq"], dtype=np.float32)
    wkv = np.asarray(inputs["wkv"], dtype=np.float32)
    wo = np.asarray(inputs["wo"], dtype=np.float32)

    # layout packs (host-side data movement + bf16 rounding)
    xT = np.ascontiguousarray(x.T.astype(bf))                       # [D, 16384]
    # wq_pack[h, p, dd*128+c] = wq[dd*128+p, h*128+c]
    wq_pack = np.ascontiguousarray(
        wq.astype(bf).reshape(ND, 128, H, 128).transpose(2, 1, 0, 3).reshape(
            H, 128, D
        )
    )
    wkv_pack = np.ascontiguousarray(
        wkv.astype(bf).reshape(ND, 128, KV // 128, 128)
        .transpose(2, 1, 0, 3).reshape(KV // 128, 128, D)
    )
    # wo_pack[j, p, hh*NJ+c] = wo[hh*128+p, j*NJ+c]
    wo_pack = np.ascontiguousarray(
        wo.astype(bf).reshape(H, 128, D // NJ, NJ).transpose(2, 1, 0, 3).reshape(
            D // NJ, 128, H * NJ
        )
    )
    anw = np.ascontiguousarray(
        np.broadcast_to(
            np.asarray(inputs["attn_norm_w"], dtype=np.float32).reshape(1, D),
            (128, D),
        ).astype(bf)
    )
    onw = np.ascontiguousarray(
        np.broadcast_to(
            np.asarray(inputs["out_norm_w"], dtype=np.float32).reshape(1, D),
            (128, D),
        ).astype(bf)
    )
    ident = np.eye(128, dtype=np.float32).astype(bf)
    return [
        {
            "xT": np.ascontiguousarray(xT[:, i * TOK : (i + 1) * TOK]),
            "wq_pack": wq_pack,
            "wkv_pack": wkv_pack,
            "wo_pack": wo_pack,
            "attn_norm_w": anw,
            "out_norm_w": onw,
            "ident": ident,
        }
        for i in range(NCORES)
    ]


def kernel(**inputs):
    from concourse.bass_utils import run_bass_kernel_spmd

    if "nc" not in _CACHE:
        _CACHE["nc"] = _build()
    nc = _CACHE["nc"]
    res = run_bass_kernel_spmd(nc, _in_maps(inputs), core_ids=list(range(NCORES)))
    out = np.concatenate(
        [res.results[i]["out"] for i in range(NCORES)], axis=0
    ).astype(np.float32)
    return out



# revision 5
# speedup vs baseline: 1.6045x; 1.6045x over previous
"""AdaptivePoolingAttention on 8 TRN2 NeuronCores — reordered attention.

Data-parallel over segments (4 whole segments per core, attention is
block-diagonal per segment); weights replicated; no collectives.

Key reordering: R=64 queries << L=512 keys per segment, so K and V are
never materialized.  With q' = (q @ wk^T):
  scores^T = x-chunks^T-contracted with q'^T       [contract D]
  p^T      = x-chunks contracted with softmax^T    [contract tokens]
  o^T      = wv-chunks contracted with p^T         [contract D]
This cuts per-core PE work ~4.8M -> ~3.3M cycles and removes the K/V
DRAM spill and all attention-side transposes (scores come out already
token-major; softmax reduces over partitions via a ones-matmul).

Host wrapper pre-packs layouts (pure data movement + bf16 rounding):
  xT       [D, TOK]            x transposed (stage A pooling + scores chunks)
  xN_pack  [SEGC, 8, 128, 2048] x natural, swizzled per (seg, dd-quad)
  wq_pack  [H, 128, D]         wq_pack[h, p, dd*128+c] = wq[dd*128+p, h*128+c]
  wkT_pack [H, 128, D]         wkT_pack[h, p, d]       = wk[d, h*128+p]
  wvh_pack [H, 128, D]         wvh_pack[h, p, dd*128+c] = wv[dd*128+p, h*128+c]
  wo_pack  [D/512, 128, H*512] wo_pack[j, p, hh*512+c] = wo[hh*128+p, j*512+c]
"""

import sys

sys.path.insert(0, "/opt/trn_rl_repo")

import numpy as np
from contextlib import ExitStack

SEG, L, R, POOL, H, HD, D, EPS = 32, 512, 64, 8, 32, 128, 4096, 1e-5
NCORES = 8
SEGC = SEG // NCORES          # 4 segments per core
TOK = SEGC * L                # 2048 tokens per core
Q = SEGC * R                  # 256 queries per core
ND = D // 128                 # 32 contraction chunks
G = 4                         # head groups
HG = H // G                   # 8 heads per group
NJ = 512                      # wo col-tile width

_CACHE = {}


def _build():
    import concourse.bass as bass
    import concourse.mybir as mybir
    import concourse.tile as tile
    from concourse import bacc

    f32 = mybir.dt.float32
    bf16 = mybir.dt.bfloat16
    ts = bass.ts
    ds = bass.ds
    AF = mybir.ActivationFunctionType
    ALU = mybir.AluOpType

    nc = bacc.Bacc("TRN2", target_bir_lowering=False, debug=False)

    xT_e = nc.declare_dram_parameter("xT", [D, TOK], bf16, isOutput=False)
    xN_e = nc.declare_dram_parameter(
        "xN_pack", [SEGC, 8, 128, 2048], bf16, isOutput=False
    )
    wq_e = nc.declare_dram_parameter("wq_pack", [H, 128, D], bf16, isOutput=False)
    wk_e = nc.declare_dram_parameter("wkT_pack", [H, 128, D], bf16, isOutput=False)
    wv_e = nc.declare_dram_parameter("wvh_pack", [H, 128, D], bf16, isOutput=False)
    wo_e = nc.declare_dram_parameter(
        "wo_pack", [D // NJ, 128, H * NJ], bf16, isOutput=False
    )
    anw_e = nc.declare_dram_parameter("attn_norm_w", [128, D], bf16, isOutput=False)
    onw_e = nc.declare_dram_parameter("out_norm_w", [128, D], bf16, isOutput=False)
    id_e = nc.declare_dram_parameter("ident", [128, 128], bf16, isOutput=False)
    out_e = nc.declare_dram_parameter("out", [Q, D], f32, isOutput=True)

    qspill_d = nc.dram_tensor("q_spill", [2, 128, D], bf16)
    oT_d = nc.dram_tensor("oT_spill", [128, H, Q], bf16)

    with tile.TileContext(nc, pool_alloc_mode="queue") as tc, ExitStack() as st:
        # ---- constants ------------------------------------------------
        cst = st.enter_context(tc.tile_pool(name="const", bufs=1))
        ident = cst.tile([128, 128], bf16)
        nc.sync.dma_start(ident[:], id_e[:])
        epst = cst.tile([128, 1], f32)
        nc.vector.memset(epst[:], EPS)
        ones = cst.tile([128, 1], bf16)
        nc.vector.memset(ones[:], 1.0)

        # qT persists stage C .. last fold; oT persists main loop .. F
        qTp = st.enter_context(tc.tile_pool(name="qTp", bufs=1))
        qT = qTp.tile([128, H, Q], bf16)

        # ---- stages A+B: pooled queries -> rmsnorm -> qnT -------------
        with tc.tile_pool(name="qnTp", bufs=1) as qnT_p:
            qnT = qnT_p.tile([128, ND, Q], bf16)
            with (
                tc.tile_pool(name="xa", bufs=2) as xa_p,
                tc.tile_pool(name="qn", bufs=1) as qn_p,
                tc.tile_pool(name="pstg", bufs=3) as pstg_p,
                tc.tile_pool(name="aps", bufs=3, space="PSUM") as aps,
            ):
                q_nat = [
                    qn_p.tile([128, D], bf16, tag=f"qnat{i}", name=f"qnat{i}")
                    for i in range(2)
                ]
                for dblk in range(ND):
                    xac = xa_p.tile([128, TOK], bf16, tag="xac")
                    nc.sync.dma_start(xac[:], xT_e[ds(dblk * 128, 128), :])
                    qtrf = pstg_p.tile([128, Q], f32, tag="qtrf")
                    nc.vector.tensor_reduce(
                        qtrf[:],
                        xac[:].rearrange("p (q e) -> p q e", e=POOL),
                        axis=mybir.AxisListType.X, op=ALU.add,
                    )
                    qtrb = pstg_p.tile([128, Q], bf16, tag="qtrb")
                    nc.vector.tensor_scalar_mul(qtrb[:], qtrf[:], 1.0 / POOL)
                    for qt in range(2):
                        pt2 = aps.tile([128, 128], bf16, tag="pt2")
                        nc.tensor.transpose(pt2[:], qtrb[:, ts(qt, 128)], ident[:])
                        nc.any.tensor_copy(q_nat[qt][:, ts(dblk, 128)], pt2[:])

                # stage B: rmsnorm(queries) -> qnT; spill raw queries
                bw_attn = qn_p.tile([128, D], bf16, tag="bwa")
                nc.scalar.dma_start(bw_attn[:], anw_e[:])
                for qt in range(2):
                    nc.gpsimd.dma_start(qspill_d[qt], q_nat[qt][:])
                    qnn = qn_p.tile([128, D], bf16, tag="qnn")
                    ssq = qn_p.tile([128, 1], f32, tag="ssq")
                    nc.scalar.activation(
                        qnn[:], q_nat[qt][:], AF.Square, accum_out=ssq[:]
                    )
                    srt = qn_p.tile([128, 1], f32, tag="srt")
                    nc.scalar.activation(
                        srt[:], ssq[:], AF.Sqrt, bias=epst[:], scale=1.0 / D
                    )
                    rs = qn_p.tile([128, 1], f32, tag="rs")
                    nc.vector.reciprocal(rs[:], srt[:])
                    nc.vector.tensor_scalar_mul(qnn[:], q_nat[qt][:], rs[:])
                    nc.vector.tensor_tensor(
                        qnn[:], qnn[:], bw_attn[:], op=ALU.mult
                    )
                    for dblk in range(ND):
                        pt = aps.tile([128, 128], bf16, tag="pt2")
                        nc.tensor.transpose(pt[:], qnn[:, ts(dblk, 128)], ident[:])
                        nc.any.tensor_copy(qnT[:, dblk, ts(qt, 128)], pt[:])

            # ---- stage C: qT = wq-proj(qnT), scale folded -------------
            with (
                tc.tile_pool(name="wqb", bufs=2) as wqb_p,
                tc.tile_pool(name="cps", bufs=2, space="PSUM") as cps,
            ):
                for h in range(H):
                    wqb = wqb_p.tile([128, ND, 128], bf16, tag="wqb")
                    nc.scalar.dma_start(
                        wqb[:], wq_e[h].rearrange("p (dd c) -> p dd c", c=128)
                    )
                    psq = cps.tile([128, Q], f32, tag="psq")
                    for dblk in range(ND):
                        nc.tensor.matmul(
                            psq[:], wqb[:, dblk, :], qnT[:, dblk, :],
                            start=(dblk == 0), stop=(dblk == ND - 1),
                        )
                    nc.scalar.mul(qT[:, h, :], psq[:], float(HD) ** -0.5)

        # ---- main loop: fold -> scores -> softmax -> pT -> o-fold -----
        with (
            tc.tile_pool(name="qp2", bufs=1) as qp2_p,
            tc.tile_pool(name="pt2", bufs=1) as pt2_p,
            tc.tile_pool(name="esb", bufs=2) as esb_p,
            tc.tile_pool(name="ost", bufs=2) as ost_p,
            tc.tile_pool(name="rcp", bufs=2) as rcp_p,
            tc.tile_pool(name="wkc", bufs=3) as wkc_p,
            tc.tile_pool(name="wvc", bufs=3) as wvc_p,
            tc.tile_pool(name="xtc", bufs=2) as xtc_p,
            tc.tile_pool(name="xnc", bufs=2) as xnc_p,
            tc.tile_pool(name="scps", bufs=4, space="PSUM") as scps,
            tc.tile_pool(name="smps", bufs=1, space="PSUM") as smps,
            tc.tile_pool(name="bigps", bufs=2, space="PSUM") as bigps,
            tc.tile_pool(name="otps", bufs=1, space="PSUM") as otps,
        ):
            qpT2 = qp2_p.tile([128, ND, HG * 128], bf16)    # 64 KiB/part
            pT2 = pt2_p.tile([128, ND, 2 * 512], bf16)      # 64 KiB/part

            def emit_fold(g, sp):
                """q'T for heads of group g, queries of seg-pair sp."""
                for hl in range(HG):
                    h = g * HG + hl
                    halves = []
                    for hf in range(2):
                        wkc = wkc_p.tile([128, D // 2], bf16, tag="wkc")
                        nc.scalar.dma_start(
                            wkc[:], wk_e[h][:, ts(hf, D // 2)]
                        )
                        halves.append(wkc)
                    for ddq in range(ND // 4):
                        fps = bigps.tile([128, 512], f32, tag="big")
                        for j in range(4):
                            dd = ddq * 4 + j
                            wkc = halves[dd // 16]
                            nc.tensor.matmul(
                                fps[:, ts(j, 128)],
                                wkc[:, ts(dd % 16, 128)],
                                qT[:, h, ts(sp, 128)],
                                start=True, stop=True,
                            )
                        nc.any.tensor_copy(
                            qpT2[:, ds(ddq * 4, 4), ds(hl * 128, 128)],
                            fps[:].rearrange("p (j c) -> p j c", c=128),
                        )

            def emit_scores(s):
                """scores^T psums for seg s; returns 4 tokblk psum tiles."""
                sc = [scps.tile([128, 512], f32, tag="sc", name=f"sc{tb}") for tb in range(4)]
                for ddq in range(ND // 4):
                    xtc = xtc_p.tile([128, 4, 512], bf16, tag="xtc")
                    nc.sync.dma_start(
                        xtc[:],
                        xT_e[ds(ddq * 512, 512), ts(s, 512)].rearrange(
                            "(d p) t -> p d t", p=128
                        ),
                    )
                    for j in range(4):
                        dd = ddq * 4 + j
                        si = s % 2
                        rhs = qpT2[:, dd, :].rearrange(
                            "p (hl c) -> p hl c", c=128
                        )[:, :, ds(si * 64, 64)]
                        for tb in range(4):
                            nc.tensor.matmul(
                                sc[tb][:], xtc[:, j, ts(tb, 128)], rhs,
                                start=(dd == 0), stop=(dd == ND - 1),
                            )
                return sc

            def emit_softmax(s, sc):
                """exp, column-sums, reciprocal broadcast, normalize."""
                si = s % 2
                esb = esb_p.tile([128, 4, 512], bf16, tag="esb")
                for tb in range(4):
                    nc.scalar.activation(esb[:, tb, :], sc[tb][:], AF.Exp)
                sm = smps.tile([1, 512], f32, tag="sm")
                for tb in range(4):
                    nc.tensor.matmul(
                        sm[:], ones[:], esb[:, tb, :],
                        start=(tb == 0), stop=(tb == 3),
                    )
                rcp = rcp_p.tile([1, 512], f32, tag="rcp")
                nc.vector.reciprocal(rcp[:], sm[:])
                rcpb = rcp_p.tile([128, 512], f32, tag="rcpb")
                nc.gpsimd.partition_broadcast(rcpb[:], rcp[:])
                for tb in range(4):
                    nc.vector.tensor_tensor(
                        esb[:, tb, :], esb[:, tb, :], rcpb[:], op=ALU.mult
                    )
                return esb

            def emit_pT(s, attnT):
                """pT2[:, :, si] = x-natural-chunks^T-contracted with attnT."""
                si = s % 2
                for ddq in range(ND // 4):
                    xnc = xnc_p.tile([128, 16, 128], bf16, tag="xnc")
                    nc.gpsimd.dma_start(
                        xnc[:],
                        xN_e[s, ddq].rearrange("p (f c) -> p f c", c=128),
                    )
                    for j in range(4):
                        dd = ddq * 4 + j
                        pps = bigps.tile([128, 512], f32, tag="big")
                        for tb in range(4):
                            nc.tensor.matmul(
                                pps[:], xnc[:, tb * 4 + j, :], attnT[:, tb, :],
                                start=(tb == 0), stop=(tb == 3),
                            )
                        nc.any.tensor_copy(pT2[:, dd, ts(si, 512)], pps[:])

            def emit_ofold(g, sp):
                """oT[:, h, sp] = wv-chunks^T-contracted with pT2."""
                for hl in range(HG):
                    h = g * HG + hl
                    halves = []
                    for hf in range(2):
                        wvc = wvc_p.tile([128, D // 2], bf16, tag="wvc")
                        nc.scalar.dma_start(
                            wvc[:], wv_e[h][:, ts(hf, D // 2)]
                        )
                        halves.append(wvc)
                    ops = otps.tile([128, 128], f32, tag="ot")
                    for dd in range(ND):
                        wvc = halves[dd // 16]
                        rhs = pT2[:, dd, :].rearrange(
                            "p (si c) -> p si c", c=512
                        )[:, :, ds(hl * 64, 64)]
                        nc.tensor.matmul(
                            ops[:], wvc[:, ts(dd % 16, 128)], rhs,
                            start=(dd == 0), stop=(dd == ND - 1),
                        )
                    osb = ost_p.tile([128, 128], bf16, tag="osb")
                    nc.any.tensor_copy(osb[:], ops[:])
                    nc.gpsimd.dma_start(oT_d[:, h, ts(sp, 128)], osb[:])

            pairs = [(g, sp) for g in range(G) for sp in range(2)]
            emit_fold(*pairs[0])
            for i, (g, sp) in enumerate(pairs):
                s0, s1 = 2 * sp, 2 * sp + 1
                sc0 = emit_scores(s0)
                sc1 = emit_scores(s1)
                attnT0 = emit_softmax(s0, sc0)
                if i + 1 < len(pairs):
                    emit_fold(*pairs[i + 1])
                emit_pT(s0, attnT0)
                attnT1 = emit_softmax(s1, sc1)
                emit_pT(s1, attnT1)
                emit_ofold(g, sp)

        # ---- stage F: out = oT' @ wo + queries, final rmsnorm ---------
        with (
            tc.tile_pool(name="wob", bufs=2) as wob_p,
            tc.tile_pool(name="qrl", bufs=1) as qrl_p,
            tc.tile_pool(name="fout", bufs=1) as fout_p,
            tc.tile_pool(name="fsc", bufs=1) as fsc_p,
            tc.tile_pool(name="fps", bufs=2, space="PSUM") as fps,
        ):
            oT = qrl_p.tile([128, H, Q], bf16, tag="oTf", name="oTf")
            nc.sync.dma_start(oT[:], oT_d[:])
            q_rl = [
                qrl_p.tile([128, D], bf16, tag=f"qrl{i}", name=f"qrl{i}")
                for i in range(2)
            ]
            for qt in range(2):
                nc.gpsimd.dma_start(q_rl[qt][:], qspill_d[qt])
            bw_out = fsc_p.tile([128, D], bf16, tag="bwo")
            nc.scalar.dma_start(bw_out[:], onw_e[:])
            out_sb = [
                fout_p.tile([128, D], bf16, tag=f"osb{i}", name=f"osb{i}")
                for i in range(2)
            ]
            for j in range(D // NJ):
                wob = wob_p.tile([128, H, NJ], bf16, tag="wob", name="wob")
                nc.scalar.dma_start(
                    wob[:], wo_e[j].rearrange("p (hh c) -> p hh c", c=NJ)
                )
                for qt in range(2):
                    ps = fps.tile([128, NJ], f32, tag="fp")
                    for hh in range(H):
                        nc.tensor.matmul(
                            ps[:], oT[:, hh, ts(qt, 128)], wob[:, hh, :],
                            start=(hh == 0), stop=(hh == H - 1),
                        )
                    nc.vector.tensor_tensor(
                        out_sb[qt][:, ts(j, NJ)], ps[:],
                        q_rl[qt][:, ts(j, NJ)], op=ALU.add,
                    )
            for qt in range(2):
                scrh = fsc_p.tile([128, D // 2], bf16, tag="fscr")
                ssq = fsc_p.tile([128, 1], f32, tag="fssq")
                ssqa = fsc_p.tile([128, 1], f32, tag="fssqa")
                for half in range(2):
                    nc.scalar.activation(
                        scrh[:], out_sb[qt][:, ts(half, D // 2)],
                        AF.Square, accum_out=(ssq if half else ssqa)[:],
                    )
                nc.vector.tensor_tensor(ssq[:], ssq[:], ssqa[:], op=ALU.add)
                srt = fsc_p.tile([128, 1], f32, tag="fsrt")
                nc.scalar.activation(
                    srt[:], ssq[:], AF.Sqrt, bias=epst[:], scale=1.0 / D
                )
                rs = fsc_p.tile([128, 1], f32, tag="frs")
                nc.vector.reciprocal(rs[:], srt[:])
                for half in range(2):
                    fin = fsc_p.tile([128, D // 2], f32, tag="ffin")
                    nc.vector.tensor_scalar_mul(
                        fin[:], out_sb[qt][:, ts(half, D // 2)], rs[:]
                    )
                    nc.vector.tensor_tensor(
                        fin[:], fin[:], bw_out[:, ts(half, D // 2)],
                        op=ALU.mult,
                    )
                    nc.sync.dma_start(
                        out_e[ts(qt, 128), ts(half, D // 2)], fin[:]
                    )

    nc.finalize()
    return nc


def _in_maps(inputs):
    import ml_dtypes

    bf = ml_dtypes.bfloat16
    x = np.asarray(inputs["x"], dtype=np.float32)
    wq = np.asarray(inputs["wq"], dtype=np.float32)
    wkv = np.asarray(inputs["wkv"], dtype=np.float32)
    wo = np.asarray(inputs["wo"], dtype=np.float32)
    wk, wv = wkv[:, : H * HD], wkv[:, H * HD :]

    wq_pack = np.ascontiguousarray(
        wq.astype(bf).reshape(ND, 128, H, 128).transpose(2, 1, 0, 3).reshape(
            H, 128, D
        )
    )
    wkT_pack = np.ascontiguousarray(
        wk.astype(bf).reshape(D, H, 128).transpose(1, 2, 0)
    )
    wvh_pack = np.ascontiguousarray(
        wv.astype(bf).reshape(ND, 128, H, 128).transpose(2, 1, 0, 3).reshape(
            H, 128, D
        )
    )
    wo_pack = np.ascontiguousarray(
        wo.astype(bf).reshape(H, 128, D // NJ, NJ).transpose(2, 1, 0, 3).reshape(
            D // NJ, 128, H * NJ
        )
    )
    anw = np.ascontiguousarray(
        np.broadcast_to(
            np.asarray(inputs["attn_norm_w"], dtype=np.float32).reshape(1, D),
            (128, D),
        ).astype(bf)
    )
    onw = np.ascontiguousarray(
        np.broadcast_to(
            np.asarray(inputs["out_norm_w"], dtype=np.float32).reshape(1, D),
            (128, D),
        ).astype(bf)
    )
    ident = np.eye(128, dtype=np.float32).astype(bf)

    maps = []
    for i in range(NCORES):
        xc = x[i * TOK : (i + 1) * TOK].astype(bf)     # [TOK, D]
        xT = np.ascontiguousarray(xc.T)                # [D, TOK]
        # xN_pack[s, ddq, p, (tb*4+dsub)*128+c] = xc[s*512+tb*128+p, (ddq*4+dsub)*128+c]
        xN = np.ascontiguousarray(
            xc.reshape(SEGC, 4, 128, ND // 4, 4, 128)
            .transpose(0, 3, 2, 1, 4, 5)
            .reshape(SEGC, ND // 4, 128, 2048)
        )
        maps.append(
            {
                "xT": xT,
                "xN_pack": xN,
                "wq_pack": wq_pack,
                "wkT_pack": wkT_pack,
                "wvh_pack": wvh_pack,
                "wo_pack": wo_pack,
                "attn_norm_w": anw,
                "out_norm_w": onw,
                "ident": ident,
            }
        )
    return maps


def kernel(**inputs):
    from concourse.bass_utils import run_bass_kernel_spmd

    if "nc" not in _CACHE:
        _CACHE["nc"] = _build()
    nc = _CACHE["nc"]
    res = run_bass_kernel_spmd(nc, _in_maps(inputs), core_ids=list(range(NCORES)))
    out = np.concatenate(
        [res.results[i]["out"] for i in range(NCORES)], axis=0
    ).astype(np.float32)
    return out
